# revision 1
# baseline (speedup 1.0000x reference)
"""DAV_Block cost-volume kernel for Trainium2 (8 NeuronCores, SPMD).

Computes sim[b,d,h,w] = cosine similarity between 3x3xC patches of q and
warped_feat[..., d]:
    qq  = box3(sum_c q^2);  kk = box3(sum_c wf_d^2);  num = box3(sum_c q*wf_d)
    sim = num / (max(sqrt(qq),eps) * max(sqrt(kk),eps))

Sharding: 8 cores = b(2) x h-quarter(4).  Each core gets a 48-row h-slice
(+1 halo row each side, zeros at global edges) with all C, W, D.

Per-core dataflow (fp32 in, fp32r through the PE):
  partitions = (h-pair, c) = 128
  ACT: sq = wf^2          -> fp32r
  DVE: pr = wf * q_bcast  -> fp32r
  PE : banded ones lhsT [128, 48] performs channel-sum AND the 3-tap h-box
       in one accumulation chain (25 h-pair matmuls per psum bank, M=48).
  Stage [128p = (w-half, h)] : 3-tap w-box via free-dim shifted adds,
       then sqrt + reciprocal_approx_fast normalization.
"""
import numpy as np
from contextlib import ExitStack

import concourse.bass as bass
from concourse import bacc
import concourse.tile as tile
from concourse import mybir
from concourse.bass_utils import run_bass_kernel_spmd

# Problem shape (hardcoded per contest contract)
B, C, H, W, D = 2, 64, 192, 320, 32
NCORES = 8
HQ = 4                 # h-quarters per batch
HOUT = H // HQ         # 48 out rows per core
HIN = HOUT + 2         # 50 input rows (1 halo each side)
NT = HIN // 2          # 25 h-pairs
J0 = HOUT              # center col of the banded weight pattern
GW = 2 * HOUT          # G width: cols [0, 96)
WBLK = 32              # w columns per main-loop tile
NWB = W // WBLK        # 10
WHALF = W // 2         # 160
FSTG = (WHALF + 2) * D  # stage free size incl. 1 halo col each side: 162*32
FVAL = WHALF * D        # 5120

_NC_CACHE = None


def _build_nc():
    nc = bacc.Bacc(None, target_bir_lowering=False)
    wf_d = nc.declare_dram_parameter("wf", [C, HIN, W, D], mybir.dt.float32, isOutput=False)
    q_d = nc.declare_dram_parameter("q", [C, HIN, W], mybir.dt.float32, isOutput=False)
    g_d = nc.declare_dram_parameter("g", [128, GW], mybir.dt.float32, isOutput=False)
    o_d = nc.declare_dram_parameter("o", [HOUT, W, D], mybir.dt.float32, isOutput=True)

    f32 = mybir.dt.float32
    f32r = mybir.dt.float32r
    SQ = mybir.ActivationFunctionType.Square

    with ExitStack() as ctx:
        tc = ctx.enter_context(tile.TileContext(nc))
        cpool = ctx.enter_context(tc.tile_pool(name="const", bufs=1))
        spool = ctx.enter_context(tc.tile_pool(name="stage", bufs=1))
        pool = ctx.enter_context(tc.tile_pool(name="work", bufs=3))
        epool = ctx.enter_context(tc.tile_pool(name="elem", bufs=2))

        # ---- constants ----
        g = cpool.tile([128, GW], f32)
        nc.sync.dma_start(g[:], g_d[:])
        gr = cpool.tile([128, GW], f32r)
        nc.scalar.copy(gr[:], g[:])
        gb = cpool.tile([128, GW], mybir.dt.bfloat16)
        nc.scalar.copy(gb[:], g[:])

        # q resident: [128p=(hpair,c), t, w]
        q_all = cpool.tile([128, NT, W], f32)
        nc.sync.dma_start(q_all[:], q_d[:].rearrange("c (t hp) w -> hp c t w", hp=2))

        # ---- stage tensors ----
        num_stg = spool.tile([112, FSTG], f32)
        kk_stg = spool.tile([112, FSTG], f32)
        qq_stg = spool.tile([112, WHALF + 2], f32)

        # ---- phase A: qq = box_h(sum_c q^2) ----
        with tc.tile_pool(name="qq_psum", bufs=1, space="PSUM") as qpsum:
            # bf16 matmuls here: fp32/fp32r matmuls each burn 64 ticks of a
            # 16-bit weight-load semaphore, capping a program at ~1023 of
            # them — the main loop needs all 1000.  bf16 keeps qq's error
            # ~1e-4, well under the output tolerance.
            qq_acc = qpsum.tile([128, W], f32)
            for t in range(NT):
                sqq = epool.tile([128, W], mybir.dt.bfloat16, tag="sqq")
                nc.scalar.activation(sqq[:], q_all[:, t, :], SQ)
                nc.tensor.matmul(
                    qq_acc[0:HOUT, :],
                    gb[:, J0 - 2 * t : J0 - 2 * t + HOUT],
                    sqq[:],
                    start=(t == 0),
                    stop=(t == NT - 1),
                )
            nc.scalar.copy(qq_stg[0:HOUT, 1 : WHALF + 1], qq_acc[0:HOUT, 0:WHALF])
            nc.scalar.copy(qq_stg[64 : 64 + HOUT, 1 : WHALF + 1], qq_acc[0:HOUT, WHALF:W])

        # ---- phase B: num/kk main loop ----
        with tc.tile_pool(name="mm_psum", bufs=2, space="PSUM") as mpsum:
            for wb in range(NWB):
                acc_num = mpsum.tile([128, 2 * 512], f32, tag="acc_num")
                acc_kk = mpsum.tile([128, 2 * 512], f32, tag="acc_kk")
                for t in range(NT):
                    wf_t = pool.tile([128, WBLK * D], f32, tag="wf")
                    src = (
                        wf_d[:]
                        .rearrange("c (t hp) w d -> t hp c w d", hp=2)[t]
                        [:, :, wb * WBLK : (wb + 1) * WBLK, :]
                    )
                    # alternate HWDGE queues (SP/ACT) — each 16-bit queue
                    # semaphore only has headroom for ~255 DMAs per run
                    dma_eng = nc.sync if (wb * NT + t) % 2 == 0 else nc.scalar
                    dma_eng.dma_start(
                        wf_t[:].rearrange("p (w d) -> p w d", d=D), src
                    )

                    sq_t = epool.tile([128, WBLK * D], f32r, tag="sq")
                    nc.scalar.activation(sq_t[:], wf_t[:], SQ)

                    pr_t = epool.tile([128, WBLK * D], f32r, tag="pr")
                    q_b = (
                        q_all[:, t, wb * WBLK : (wb + 1) * WBLK]
                        .unsqueeze(-1)
                        .broadcast_to([128, WBLK, D])
                    )
                    nc.vector.tensor_mul(
                        pr_t[:].rearrange("p (w d) -> p w d", d=D),
                        wf_t[:].rearrange("p (w d) -> p w d", d=D),
                        q_b,
                    )

                    lhsT = gr[:, J0 - 2 * t : J0 - 2 * t + HOUT]
                    first, last = (t == 0), (t == NT - 1)
                    for ch in range(2):
                        sl = slice(512 * ch, 512 * (ch + 1))
                        nc.tensor.matmul(acc_num[0:HOUT, sl], lhsT, pr_t[:, sl],
                                         start=first, stop=last)
                        nc.tensor.matmul(acc_kk[0:HOUT, sl], lhsT, sq_t[:, sl],
                                         start=first, stop=last)

                # evacuate this w-block: psum [48, 1024] -> stage quadrant
                wg, wo = wb // (NWB // 2), (wb % (NWB // 2)) * WBLK
                pbase = 64 * wg
                foff = (1 + wo) * D
                nc.scalar.copy(
                    num_stg[pbase : pbase + HOUT, foff : foff + 1024],
                    acc_num[0:HOUT, :],
                )
                nc.vector.tensor_copy(
                    kk_stg[pbase : pbase + HOUT, foff : foff + 1024],
                    acc_kk[0:HOUT, :],
                )

        # ---- phase C: halos, box-w, normalize ----
        # zero halos at global w edges
        nc.gpsimd.memset(num_stg[0:HOUT, 0:D], 0.0)
        nc.gpsimd.memset(kk_stg[0:HOUT, 0:D], 0.0)
        nc.gpsimd.memset(qq_stg[0:HOUT, 0:1], 0.0)
        nc.gpsimd.memset(num_stg[64 : 64 + HOUT, (WHALF + 1) * D : FSTG], 0.0)
        nc.gpsimd.memset(kk_stg[64 : 64 + HOUT, (WHALF + 1) * D : FSTG], 0.0)
        nc.gpsimd.memset(qq_stg[64 : 64 + HOUT, WHALF + 1 : WHALF + 2], 0.0)
        # interface halos between the two w-halves (cross-quadrant copies)
        nc.scalar.copy(num_stg[0:HOUT, (WHALF + 1) * D : FSTG],
                       num_stg[64 : 64 + HOUT, D : 2 * D])
        nc.scalar.copy(num_stg[64 : 64 + HOUT, 0:D],
                       num_stg[0:HOUT, WHALF * D : (WHALF + 1) * D])
        nc.scalar.copy(kk_stg[0:HOUT, (WHALF + 1) * D : FSTG],
                       kk_stg[64 : 64 + HOUT, D : 2 * D])
        nc.scalar.copy(kk_stg[64 : 64 + HOUT, 0:D],
                       kk_stg[0:HOUT, WHALF * D : (WHALF + 1) * D])
        nc.scalar.copy(qq_stg[0:HOUT, WHALF + 1 : WHALF + 2],
                       qq_stg[64 : 64 + HOUT, 1:2])
        nc.scalar.copy(qq_stg[64 : 64 + HOUT, 0:1],
                       qq_stg[0:HOUT, WHALF : WHALF + 1])

        # box-w (3-tap along w = free-dim shifts by D)
        box_num = spool.tile([112, FVAL], f32)
        box_kk = spool.tile([112, FVAL], f32)
        qq_box = spool.tile([112, WHALF], f32)
        nc.vector.tensor_add(box_num[0:112, :], num_stg[0:112, 0:FVAL],
                             num_stg[0:112, 2 * D : FVAL + 2 * D])
        nc.vector.tensor_add(box_num[0:112, :], box_num[0:112, :],
                             num_stg[0:112, D : FVAL + D])
        nc.vector.tensor_add(box_kk[0:112, :], kk_stg[0:112, 0:FVAL],
                             kk_stg[0:112, 2 * D : FVAL + 2 * D])
        nc.vector.tensor_add(box_kk[0:112, :], box_kk[0:112, :],
                             kk_stg[0:112, D : FVAL + D])
        nc.vector.tensor_add(qq_box[0:112, :], qq_stg[0:112, 0:WHALF],
                             qq_stg[0:112, 2 : WHALF + 2])
        nc.vector.tensor_add(qq_box[0:112, :], qq_box[0:112, :],
                             qq_stg[0:112, 1 : WHALF + 1])

        # normalize: sim = box_num * recip(sqrt(box_kk * qq_box))
        prod = kk_stg  # reuse
        nc.vector.tensor_mul(
            prod[0:112, 0:FVAL].rearrange("p (w d) -> p w d", d=D),
            box_kk[0:112, :].rearrange("p (w d) -> p w d", d=D),
            qq_box[0:112, :].unsqueeze(-1).broadcast_to([112, WHALF, D]),
        )
        s = num_stg  # reuse
        nc.scalar.activation(s[0:112, 0:FVAL], prod[0:112, 0:FVAL],
                             mybir.ActivationFunctionType.Sqrt)
        r = prod  # reuse again
        nc.vector.reciprocal_approx_fast(r[0:112, 0:FVAL], s[0:112, 0:FVAL])
        sim = box_kk  # reuse
        nc.vector.tensor_mul(sim[0:112, :], box_num[0:112, :], r[0:112, 0:FVAL])

        # ---- output ----
        nc.sync.dma_start(
            o_d[:, 0:WHALF, :],
            sim[0:HOUT, :].rearrange("p (w d) -> p w d", d=D),
        )
        nc.sync.dma_start(
            o_d[:, WHALF:W, :],
            sim[64 : 64 + HOUT, :].rearrange("p (w d) -> p w d", d=D),
        )

    nc.compile()
    return nc


def _g_pattern() -> np.ndarray:
    """g[p=(hp*64+c), j] = 1 iff j - J0 in {hp-2, hp-1, hp}."""
    g = np.zeros((128, GW), dtype=np.float32)
    for hp in range(2):
        for dj in (hp - 2, hp - 1, hp):
            j = J0 + dj
            if 0 <= j < GW:
                g[hp * 64 : (hp + 1) * 64, j] = 1.0
    return g


def get_nc():
    global _NC_CACHE
    if _NC_CACHE is None:
        _NC_CACHE = _build_nc()
    return _NC_CACHE


def make_in_maps(q: np.ndarray, warped_feat: np.ndarray):
    """Marshal full inputs into 8 per-core input maps."""
    q = np.asarray(q, dtype=np.float32)
    wf = np.asarray(warped_feat, dtype=np.float32)
    g = _g_pattern()
    in_maps = []
    for core in range(NCORES):
        b, j = divmod(core, HQ)
        h0 = j * HOUT - 1          # inclusive, may be -1
        h1 = j * HOUT + HOUT + 1   # exclusive, may be H+1
        lo_pad = 1 if h0 < 0 else 0
        hi_pad = 1 if h1 > H else 0
        hs = slice(h0 + lo_pad, h1 - hi_pad)
        q_c = np.zeros((C, HIN, W), dtype=np.float32)
        q_c[:, lo_pad : HIN - hi_pad, :] = q[b][:, hs, :]
        wf_c = np.zeros((C, HIN, W, D), dtype=np.float32)
        wf_c[:, lo_pad : HIN - hi_pad, :, :] = wf[b][:, hs, :, :]
        in_maps.append({"wf": wf_c, "q": q_c, "g": g})
    return in_maps


def assemble(results) -> np.ndarray:
    out = np.empty((B, D, H, W), dtype=np.float32)
    for core in range(NCORES):
        b, j = divmod(core, HQ)
        o = results[core]["o"]  # [48, 320, 32]
        out[b, :, j * HOUT : (j + 1) * HOUT, :] = o.transpose(2, 0, 1)
    return out


def kernel(q: np.ndarray, warped_feat: np.ndarray) -> np.ndarray:
    nc = get_nc()
    in_maps = make_in_maps(q, warped_feat)
    res = run_bass_kernel_spmd(nc, in_maps, list(range(NCORES)))
    return assemble(res.results)

